# revision 31
# baseline (speedup 1.0000x reference)
"""Trainium2 Bass kernel for nn_BarrierPolicy (CBF-QP safety filter).

Data-parallel over batch: 8 cores x 32768 samples.
Phase A (per 2048-sample tile): load x in "xview" layout, PE-transpose to
"SP2" (stacked pack-2) layout, run the 3-layer MLP + dynamics matmuls on the
tensor engine in bf16 (1 cycle/row), transpose results back to xview.
Bias-add for px and the sigmoid for alpha are deferred to xview where they
are single wide ops instead of many narrow ones.
Phase B (per 1024-col chunk, 2 chunks): Kiwiel variable-fixing active-set
solve of the per-sample box-QP dual (3 iterations + closed-form finish) in
bf16 elementwise / f32 slot math, then u = clip(-p + lam*g) in f32.
Broadcast of per-sample scalars over the 8 coords is materialized by the
scalar (ACT) engine; slot math mostly on GPSIMD to keep DVE for the wide
bf16 elementwise ops.

Layouts (per tile of 2048 samples):
  xview: SBUF (128, 128): partition r, col 16b+8s0+j <-> sample 256b+2r+s0, coord j
  SP2  : transpose of xview: partition 16b+8s0+j, col r
  padded psum (for 16-row matmul outs, 32-align rule): block b at partitions
  [32(b%3), 32(b%3)+16), free-slot b//3.
  slot : per-sample scalars (128, 256): partition r, col 2b+s0 per tile
"""
import numpy as np

B_FULL, N = 262144, 8
NCORES = 8
S = B_FULL // NCORES          # 32768 samples per core
TILE = 2048
NT = S // TILE                # 16 tiles
NSLOT = S // 128              # 256 slot cols per core
NCH = 4                       # phase-B chunks
TPC = NT // NCH               # tiles per chunk
FC = S // 16                  # 2048 xview cols per core
T_KIWIEL = 3
LAMCAP = float(2.0 ** 40)
EPS = 1e-12

_CACHE = {}

_CSHAPES_BF = dict(TL2=(128, 128), TL3px=(64, 16), TL3a=(128, 2),
                   TDA=(128, 128), TDG=(128, 128), ID128H=(128, 128),
                   B31J=(128, 16),
                   **{f"TL1E{b}": (128, 128) for b in range(8)})
_CSHAPES_F32 = dict(ID128=(128, 128), B1v=(128, 1), B2v=(128, 1),
                    B32s=(128, 1))


def _consts(W1, b1, W21, b21, W22, b22, W31, b31, W32, b32, A, G):
    import ml_dtypes
    f32 = np.float32
    bf = ml_dtypes.bfloat16
    out = {}
    for b in range(8):
        T = np.zeros((128, 128), f32)
        for s0 in range(2):
            T[16 * b + 8 * s0:16 * b + 8 * s0 + 8, 64 * s0:64 * s0 + 64] = W1
        out[f"TL1E{b}"] = T.astype(bf)
    TL2 = np.zeros((128, 128), f32)
    for s0 in range(2):
        TL2[64 * s0:64 * s0 + 64, 32 * s0:32 * s0 + 32] = W21
        TL2[64 * s0:64 * s0 + 64, 64 + 32 * s0:64 + 32 * s0 + 32] = W22
    TL3px = np.zeros((64, 16), f32)
    for s0 in range(2):
        TL3px[32 * s0:32 * s0 + 32, 8 * s0:8 * s0 + 8] = W31
    TL3a = np.zeros((128, 2), f32)          # used as slice [64:128)
    for s0 in range(2):
        TL3a[64 + 32 * s0:64 + 32 * s0 + 32, s0:s0 + 1] = W32
    TDA = np.kron(np.eye(16, dtype=f32), A.T.astype(f32))         # out = A x
    TDG = np.kron(np.eye(16, dtype=f32), (-2.0 * G).astype(f32))  # out = -2 G^T x
    # per-coordinate b31 bias: col 8s+j -> b31[j]
    B31J = np.tile(b31.astype(f32), 2)[None, :].repeat(128, 0)
    out.update(TL2=TL2.astype(bf), TL3px=TL3px.astype(bf), TL3a=TL3a.astype(bf),
               TDA=TDA.astype(bf), TDG=TDG.astype(bf),
               ID128H=np.eye(128, dtype=f32).astype(bf),
               B31J=B31J.astype(bf))
    out["ID128"] = np.eye(128, dtype=f32)
    out["B1v"] = np.concatenate([b1, b1]).reshape(128, 1).astype(f32)
    out["B2v"] = np.concatenate([b21, b21, b22, b22]).reshape(128, 1).astype(f32)
    out["B32s"] = np.full((128, 1), float(b32[0]), f32)
    return out


def build_kernel(nc, tc, x_d, u_d, cds):
    from concourse import mybir
    f32 = mybir.dt.float32
    f32r = mybir.dt.float32r
    bf16 = mybir.dt.bfloat16
    AL = mybir.AluOpType
    AF = mybir.ActivationFunctionType
    XL = mybir.AxisListType.X
    V, GP, SC = nc.vector, nc.gpsimd, nc.scalar

    with (
        tc.tile_pool(name="const", bufs=1) as cpool,
        tc.tile_pool(name="pers", bufs=1) as pers,
        tc.tile_pool(name="work", bufs=2) as work,
        tc.tile_pool(name="psA", bufs=1, space="PSUM") as psA,
        tc.tile_pool(name="psB", bufs=1, space="PSUM") as psB,
    ):
        C = {}
        for k, v in _CSHAPES_BF.items():
            C[k] = cpool.tile(list(v), bf16, tag=k, name=k)
        for k, v in _CSHAPES_F32.items():
            C[k] = cpool.tile(list(v), f32, tag=k, name=k)
        for k in list(_CSHAPES_BF) + list(_CSHAPES_F32):
            nc.sync.dma_start(C[k][:], cds[k][:])

        def fc_f32(tag):
            return pers.tile([128, FC], f32, tag=tag, name=tag)

        def fc_bf(tag):
            return pers.tile([128, FC], bf16, tag=tag, name=tag)

        def sl_tile(tag):
            return pers.tile([128, NSLOT], f32, tag=tag, name=tag)

        x_xv, u32 = fc_f32("x_xv"), fc_f32("u32")
        p_xv, g_xv = fc_bf("p_xv"), fc_bf("g_xv")
        gt_xv, pt_xv, q_xv = fc_bf("gt_xv"), fc_bf("pt_xv"), fc_bf("q_xv")
        zt_xv, mm_xv = fc_bf("zt_xv"), fc_bf("mm_xv")
        sc1, sc2, sc3 = fc_bf("sc1"), fc_bf("sc2"), fc_bf("sc3")
        lbc, bvbc = fc_bf("lbc"), fc_bf("bvbc")
        araw = sl_tile("araw")
        alpha4, lfhx, sxx = sl_tile("alpha4"), sl_tile("lfhx"), sl_tile("sxx")
        c0s, viol, nviol, infs = (sl_tile("c0s"), sl_tile("viol"),
                                  sl_tile("nviol"), sl_tile("infs"))
        nums, dens, lams = sl_tile("nums"), sl_tile("dens"), sl_tile("lams")
        t1s, t2s = sl_tile("t1s"), sl_tile("t2s")

        # ---------------- Phase A ----------------
        # px/alpha matmul outs leave pad regions unwritten; zero once so the
        # full-tile evac copies and transposes never see uninitialized PSUM.
        LPx = psA.tile([128, 2, 128], f32, tag="LPx", name="LPx")
        alP = psA.tile([128, 2, 128], f32, tag="alP", name="alP")
        V.memset(LPx[:], 0.0)
        V.memset(alP[:], 0.0)
        for t in range(NT):
            cs = slice(128 * t, 128 * t + 128)
            ss = slice(16 * t, 16 * t + 16)
            nc.sync.dma_start(
                x_xv[:, cs].rearrange("p (b s j) -> p b s j", b=8, s=2, j=8),
                x_d[t * TILE:(t + 1) * TILE, :].rearrange(
                    "(b r s) j -> r b s j", b=8, r=128, s=2))
            TP = psA.tile([128, 3, 128], f32, tag="TP", name="TP")
            nc.tensor.transpose(TP[:, 0, :], x_xv[:, cs], C["ID128"][:])
            xsp2 = work.tile([128, 128], bf16, tag="xsp2", name="xsp2")
            V.tensor_copy(xsp2[:], TP[:, 0, :])

            h1P = psA.tile([128, 8, 128], f32, tag="h1P", name="h1P")
            x2P = psA.tile([128, 8, 128], f32, tag="x2P", name="x2P")
            h1 = work.tile([128, 8, 128], bf16, tag="h1", name="h1")
            x2 = work.tile([128, 8, 128], bf16, tag="x2", name="x2")

            for b in range(8):
                nc.tensor.matmul(h1P[:, b, :], C[f"TL1E{b}"][:], xsp2[:])
            # relu + bias evac, one wide op (PSUM: DVE/ACT only)
            SC.activation(h1[:], h1P[:], AF.Relu, bias=C["B1v"][:])
            for b in range(8):
                nc.tensor.matmul(x2P[:, b, :], C["TL2"][:], h1[:, b, :])
            SC.activation(x2[:, 0:4, :], x2P[:, 0:4, :], AF.Relu, bias=C["B2v"][:])
            V.tensor_scalar(x2[:, 4:8, :], x2P[:, 4:8, :], C["B2v"][:], 0.0,
                            AL.add, AL.max)
            for b in range(8):
                m4, k2 = b % 4, b // 4
                nc.tensor.matmul(LPx[32 * m4:32 * m4 + 16, k2, :],
                                 C["TL3px"][:], x2[0:64, b, :],
                                 tile_position=(0, 32 * m4))
                nc.tensor.matmul(alP[32 * m4:32 * m4 + 2, k2, :],
                                 C["TL3a"][64:128, :], x2[64:128, b, :],
                                 tile_position=(64, 32 * m4))

            # raw px / alpha evac (bias+sigmoid deferred to xview)
            pxe = work.tile([128, 2, 128], bf16, tag="pxe", name="pxe")
            asle = work.tile([128, 2, 128], bf16, tag="asle", name="asle")
            SC.activation(pxe[:], LPx[:], AF.Copy)
            SC.activation(asle[:], alP[:], AF.Copy)

            nc.tensor.matmul(TP[:, 1, :], C["TDA"][:], xsp2[:])
            nc.tensor.matmul(TP[:, 2, :], C["TDG"][:], xsp2[:])
            axs = work.tile([128, 128], bf16, tag="axs", name="axs")
            gsp2 = work.tile([128, 128], bf16, tag="gsp2", name="gsp2")
            V.tensor_copy(axs[:], TP[:, 1, :])
            V.tensor_copy(gsp2[:], TP[:, 2, :])

            # transposes back to xview (single PSUM bank for all six)
            trP = psB.tile([128, 6, 128], bf16, tag="trP", name="trP")
            pxtP = trP[:, 2:4, :]
            altP = trP[:, 4:6, :]
            nc.tensor.transpose(trP[:, 0, :], gsp2[:], C["ID128H"][:])
            nc.tensor.transpose(trP[:, 1, :], axs[:], C["ID128H"][:])
            V.tensor_copy(g_xv[:, cs], trP[:, 0, :])
            prodA = work.tile([128, 128], f32, tag="prodA", name="prodA")
            V.scalar_tensor_tensor(prodA[:], trP[:, 1, :], -2.0,
                                   x_xv[:, cs], AL.mult, AL.mult)
            V.tensor_reduce(lfhx[:, ss],
                            prodA[:].rearrange("p (c j) -> p c j", j=8),
                            XL, AL.add)
            sqx = work.tile([128, 128], f32, tag="sqx", name="sqx")
            GP.tensor_tensor(sqx[:], x_xv[:, cs], x_xv[:, cs], AL.mult)
            V.tensor_reduce(sxx[:, ss],
                            sqx[:].rearrange("p (c j) -> p c j", j=8),
                            XL, AL.add)

            for k in range(2):
                nc.tensor.transpose(pxtP[:, k, :], pxe[:, k, :], C["ID128H"][:])
                nc.tensor.transpose(altP[:, k, :], asle[:, k, :], C["ID128H"][:])
            # px (+ b31 bias) and alpha back to xview, one wide op each
            dstp = p_xv[:, cs].rearrange("p (k m sj) -> p k m sj",
                                         k=2, m=4, sj=16)
            srcp = pxtP.rearrange("p k (m g sj) -> p k m g sj",
                                     m=4, g=2, sj=16)[:, :, :, 0, :]
            V.tensor_tensor(
                dstp, srcp,
                C["B31J"][:].rearrange("p (k m sj) -> p k m sj",
                                       k=1, m=1, sj=16)
                .broadcast_to((128, 2, 4, 16)), AL.add)
            dsta = araw[:, ss].rearrange("p (k m s) -> p k m s", k=2, m=4, s=2)
            srca = altP.rearrange("p k (m g) -> p k m g",
                                     m=4, g=32)[:, :, :, 0:2]
            V.tensor_copy(dsta, srca)

        # ---------------- Phase B (per chunk) ----------------
        CF = FC // NCH          # 1024 fc cols per chunk
        CL = NSLOT // NCH       # 128 slot cols per chunk
        x3 = lambda ap: ap.rearrange("p (c j) -> p c j", j=8)

        for ch in range(NCH):
            fs = slice(CF * ch, CF * ch + CF)
            sl = slice(CL * ch, CL * ch + CL)
            pF, gF = p_xv[:, fs], g_xv[:, fs]
            gtF, ptF, qF = gt_xv[:, fs], pt_xv[:, fs], q_xv[:, fs]
            ztF, mmF = zt_xv[:, fs], mm_xv[:, fs]
            s1F, s2F, s3F = sc1[:, fs], sc2[:, fs], sc3[:, fs]
            lbcF, bvbcF = lbc[:, fs], bvbc[:, fs]
            u32F = u32[:, fs]
            c0L, viL, nviL, inL = c0s[:, sl], viol[:, sl], nviol[:, sl], infs[:, sl]
            nmL, dnL, lmL = nums[:, sl], dens[:, sl], lams[:, sl]
            t1L, t2L = t1s[:, sl], t2s[:, sl]
            arL, a4L = araw[:, sl], alpha4[:, sl]
            bcv = lambda apL: apL.broadcast_to((128, CL, 8))

            # alpha sigmoid (deferred from phase A; b31 bias folded into evac)
            SC.activation(a4L, arL, AF.Sigmoid, bias=C["B32s"][:])

            # c0 = Lfhx + 4*sigm*(16 - sxx);  (alpha4 holds the sigmoid)
            GP.tensor_scalar(t1L, sxx[:, sl], -1.0, 16.0, AL.mult, AL.add)
            GP.tensor_tensor(t2L, a4L, t1L, AL.mult)
            GP.tensor_scalar(t2L, t2L, 4.0, None, AL.mult)
            GP.tensor_tensor(c0L, t2L, lfhx[:, sl], AL.add)

            # transformed QP data
            SC.sign(s1F, gF)                                   # sigma
            V.tensor_tensor(ptF, s1F, pF, AL.mult)             # pt = sigma*p
            V.tensor_scalar(ztF, ptF, -1.0, None, AL.mult)     # zt0 = -pt
            SC.activation(gtF, gF, AF.Abs)
            SC.activation(qF, gF, AF.Square)
            GP.memset(mmF, 1.0)

            # c(0) and feasibility
            V.tensor_scalar(s2F, ztF, 1.0, -1.0, AL.min, AL.max)   # u0
            V.tensor_tensor(s1F, gtF, s2F, AL.mult)
            V.tensor_reduce(t1L, x3(s1F), XL, AL.add)
            GP.tensor_tensor(t1L, c0L, t1L, AL.add)
            GP.tensor_scalar(viL, t1L, 0.0, None, AL.is_lt)
            GP.tensor_scalar(nviL, viL, -1.0, None, AL.mult)
            V.tensor_reduce(t2L, x3(gtF), XL, AL.add)
            GP.tensor_tensor(t2L, c0L, t2L, AL.add)
            GP.tensor_scalar(inL, t2L, 0.0, None, AL.is_lt)
            GP.tensor_tensor(inL, inL, viL, AL.mult)

            # initial num/den (zt = -pt, mm = 1)
            V.tensor_tensor(s1F, gtF, ztF, AL.mult)
            V.tensor_reduce(nmL, x3(s1F), XL, AL.add)
            GP.tensor_tensor(nmL, c0L, nmL, AL.add)
            V.tensor_reduce(dnL, x3(qF), XL, AL.add)

            def calc_lam():
                GP.tensor_scalar(t1L, dnL, EPS, None, AL.add)
                V.reciprocal(t2L, t1L)
                GP.tensor_tensor(lmL, nmL, t2L, AL.mult)
                GP.tensor_tensor(lmL, lmL, nviL, AL.mult)      # lam = -num/den*viol

            calc_lam()
            for _ in range(T_KIWIEL):
                SC.activation(x3(lbcF), bcv(lmL), AF.Copy)         # lam bcast
                V.tensor_tensor(s2F, lbcF, gtF, AL.mult)
                V.tensor_tensor(s2F, s2F, ptF, AL.subtract)        # ur
                V.tensor_scalar(s2F, s2F, 1.0, -1.0, AL.min, AL.max)  # uhat
                V.tensor_tensor(s1F, gtF, s2F, AL.mult)
                V.tensor_reduce(t1L, x3(s1F), XL, AL.add)
                GP.tensor_tensor(t1L, c0L, t1L, AL.add)            # c
                GP.tensor_scalar(t2L, t1L, -1.0, None, AL.mult)    # -c
                SC.activation(x3(bvbcF), bcv(t2L), AF.Sign)        # bvs = sign(-c)
                V.tensor_tensor(s1F, bvbcF, s2F, AL.mult)
                V.tensor_scalar(s1F, s1F, 1.0, None, AL.is_ge)
                V.tensor_tensor(s1F, s1F, mmF, AL.mult)            # fix
                GP.tensor_tensor(s3F, bvbcF, ztF, AL.subtract)
                V.tensor_tensor(s3F, s1F, s3F, AL.mult)
                V.tensor_tensor(ztF, ztF, s3F, AL.add)
                GP.tensor_tensor(mmF, mmF, s1F, AL.subtract)
                V.tensor_tensor(s1F, gtF, ztF, AL.mult)
                V.tensor_reduce(nmL, x3(s1F), XL, AL.add)
                GP.tensor_tensor(nmL, c0L, nmL, AL.add)
                V.tensor_tensor(s1F, qF, mmF, AL.mult)
                V.tensor_reduce(dnL, x3(s1F), XL, AL.add)
                calc_lam()

            # infeasible rows -> lam = LAMCAP
            GP.tensor_scalar(t1L, lmL, -1.0, LAMCAP, AL.mult, AL.add)
            GP.tensor_tensor(t1L, t1L, inL, AL.mult)
            GP.tensor_tensor(lmL, lmL, t1L, AL.add)
            SC.activation(x3(lbcF), bcv(lmL), AF.Copy)
            V.tensor_tensor(s1F, lbcF, gF, AL.mult)
            V.tensor_tensor(s1F, s1F, pF, AL.subtract)
            V.tensor_scalar(u32F, s1F, 1.0, -1.0, AL.min, AL.max)
            for tt in range(TPC * ch, TPC * (ch + 1)):
                nc.sync.dma_start(
                    u_d[tt * TILE:(tt + 1) * TILE, :].rearrange(
                        "(b r s) j -> r b s j", b=8, r=128, s=2),
                    u32[:, 128 * tt:128 * tt + 128].rearrange(
                        "p (b s j) -> p b s j", b=8, s=2, j=8))


def _build():
    from concourse import bacc, mybir
    from concourse import tile as tile_mod
    from concourse._compat import axon_active
    f32 = mybir.dt.float32
    bf16 = mybir.dt.bfloat16
    nc = bacc.Bacc("TRN2", target_bir_lowering=False,
                   debug=not axon_active(), num_devices=NCORES)
    x_d = nc.dram_tensor("x", [S, N], f32, kind="ExternalInput").ap()
    u_d = nc.dram_tensor("u", [S, N], f32, kind="ExternalOutput").ap()
    cds = {}
    for k, v in _CSHAPES_BF.items():
        cds[k] = nc.dram_tensor(k, list(v), bf16, kind="ExternalInput").ap()
    for k, v in _CSHAPES_F32.items():
        cds[k] = nc.dram_tensor(k, list(v), f32, kind="ExternalInput").ap()
    with tile_mod.TileContext(nc) as tc:
        build_kernel(nc, tc, x_d, u_d, cds)
    nc.compile()
    return nc


def kernel(x, W1, b1, W21, b21, W22, b22, W31, b31, W32, b32, A, G, mean, std):
    from concourse.bass_utils import run_bass_kernel_spmd
    f32 = np.float32
    x = np.asarray(x, f32)
    x0 = (x * np.asarray(std, f32) + np.asarray(mean, f32)).astype(f32)

    consts = _consts(np.asarray(W1, f32), np.asarray(b1, f32), np.asarray(W21, f32),
                     np.asarray(b21, f32), np.asarray(W22, f32), np.asarray(b22, f32),
                     np.asarray(W31, f32), np.asarray(b31, f32), np.asarray(W32, f32),
                     np.asarray(b32, f32), np.asarray(A, f32), np.asarray(G, f32))
    if "nc" not in _CACHE:
        _CACHE["nc"] = _build()
    nc = _CACHE["nc"]

    in_maps = []
    for c in range(NCORES):
        m = {"x": np.ascontiguousarray(x0[c * S:(c + 1) * S])}
        m.update(consts)
        in_maps.append(m)
    res = run_bass_kernel_spmd(nc, in_maps, list(range(NCORES)))
    out = np.concatenate([np.asarray(res.results[c]["u"]) for c in range(NCORES)],
                         axis=0)
    return out.astype(f32)


# revision 33
# speedup vs baseline: 1.0249x; 1.0249x over previous
"""Trainium2 Bass kernel for nn_BarrierPolicy (CBF-QP safety filter).

Data-parallel over batch: 8 cores x 32768 samples.
Phase A (per 2048-sample tile): load x in "xview" layout, PE-transpose to
"SP2" (stacked pack-2) layout, run the 3-layer MLP + dynamics matmuls on the
tensor engine in bf16 (1 cycle/row), transpose results back to xview.
Bias-add for px and the sigmoid for alpha are deferred to xview where they
are single wide ops instead of many narrow ones.
Phase B (per 1024-col chunk, 2 chunks): Kiwiel variable-fixing active-set
solve of the per-sample box-QP dual (3 iterations + closed-form finish) in
bf16 elementwise / f32 slot math, then u = clip(-p + lam*g) in f32.
Broadcast of per-sample scalars over the 8 coords is materialized by the
scalar (ACT) engine; slot math mostly on GPSIMD to keep DVE for the wide
bf16 elementwise ops.

Layouts (per tile of 2048 samples):
  xview: SBUF (128, 128): partition r, col 16b+8s0+j <-> sample 256b+2r+s0, coord j
  SP2  : transpose of xview: partition 16b+8s0+j, col r
  padded psum (for 16-row matmul outs, 32-align rule): block b at partitions
  [32(b%3), 32(b%3)+16), free-slot b//3.
  slot : per-sample scalars (128, 256): partition r, col 2b+s0 per tile
"""
import numpy as np

B_FULL, N = 262144, 8
NCORES = 8
S = B_FULL // NCORES          # 32768 samples per core
TILE = 2048
NT = S // TILE                # 16 tiles
NSLOT = S // 128              # 256 slot cols per core
NCH = 4                       # phase-B chunks
TPC = NT // NCH               # tiles per chunk
FC = S // 16                  # 2048 xview cols per core
T_KIWIEL = 3
LAMCAP = float(2.0 ** 40)
EPS = 1e-12

_CACHE = {}

_CSHAPES_BF = dict(TL2=(128, 128), TL3px=(64, 16), TL3a=(128, 2),
                   TDA=(128, 128), TDG=(128, 128), ID128H=(128, 128),
                   B31J=(128, 16),
                   **{f"TL1E{b}": (128, 128) for b in range(8)})
_CSHAPES_F32 = dict(ID128=(128, 128), B1v=(128, 1), B2v=(128, 1),
                    B32s=(128, 1))


def _consts(W1, b1, W21, b21, W22, b22, W31, b31, W32, b32, A, G):
    import ml_dtypes
    f32 = np.float32
    bf = ml_dtypes.bfloat16
    out = {}
    for b in range(8):
        T = np.zeros((128, 128), f32)
        for s0 in range(2):
            T[16 * b + 8 * s0:16 * b + 8 * s0 + 8, 64 * s0:64 * s0 + 64] = W1
        out[f"TL1E{b}"] = T.astype(bf)
    TL2 = np.zeros((128, 128), f32)
    for s0 in range(2):
        TL2[64 * s0:64 * s0 + 64, 32 * s0:32 * s0 + 32] = W21
        TL2[64 * s0:64 * s0 + 64, 64 + 32 * s0:64 + 32 * s0 + 32] = W22
    TL3px = np.zeros((64, 16), f32)
    for s0 in range(2):
        TL3px[32 * s0:32 * s0 + 32, 8 * s0:8 * s0 + 8] = W31
    TL3a = np.zeros((128, 2), f32)          # used as slice [64:128)
    for s0 in range(2):
        TL3a[64 + 32 * s0:64 + 32 * s0 + 32, s0:s0 + 1] = W32
    TDA = np.kron(np.eye(16, dtype=f32), A.T.astype(f32))         # out = A x
    TDG = np.kron(np.eye(16, dtype=f32), (-2.0 * G).astype(f32))  # out = -2 G^T x
    # per-coordinate b31 bias: col 8s+j -> b31[j]
    B31J = np.tile(b31.astype(f32), 2)[None, :].repeat(128, 0)
    out.update(TL2=TL2.astype(bf), TL3px=TL3px.astype(bf), TL3a=TL3a.astype(bf),
               TDA=TDA.astype(bf), TDG=TDG.astype(bf),
               ID128H=np.eye(128, dtype=f32).astype(bf),
               B31J=B31J.astype(bf))
    out["ID128"] = np.eye(128, dtype=f32)
    out["B1v"] = np.concatenate([b1, b1]).reshape(128, 1).astype(f32)
    out["B2v"] = np.concatenate([b21, b21, b22, b22]).reshape(128, 1).astype(f32)
    out["B32s"] = np.full((128, 1), float(b32[0]), f32)
    return out


def build_kernel(nc, tc, x_d, u_d, cds):
    from concourse import mybir
    f32 = mybir.dt.float32
    f32r = mybir.dt.float32r
    bf16 = mybir.dt.bfloat16
    AL = mybir.AluOpType
    AF = mybir.ActivationFunctionType
    XL = mybir.AxisListType.X
    V, GP, SC = nc.vector, nc.gpsimd, nc.scalar

    with (
        tc.tile_pool(name="const", bufs=1) as cpool,
        tc.tile_pool(name="pers", bufs=1) as pers,
        tc.tile_pool(name="work", bufs=2) as work,
        tc.tile_pool(name="psA", bufs=1, space="PSUM") as psA,
        tc.tile_pool(name="psB", bufs=1, space="PSUM") as psB,
    ):
        C = {}
        for k, v in _CSHAPES_BF.items():
            C[k] = cpool.tile(list(v), bf16, tag=k, name=k)
        for k, v in _CSHAPES_F32.items():
            C[k] = cpool.tile(list(v), f32, tag=k, name=k)
        for k in list(_CSHAPES_BF) + list(_CSHAPES_F32):
            nc.sync.dma_start(C[k][:], cds[k][:])

        def fc_f32(tag):
            return pers.tile([128, FC], f32, tag=tag, name=tag)

        def fc_bf(tag):
            return pers.tile([128, FC], bf16, tag=tag, name=tag)

        def sl_tile(tag):
            return pers.tile([128, NSLOT], f32, tag=tag, name=tag)

        x_xv, u32 = fc_f32("x_xv"), fc_f32("u32")
        p_xv, g_xv = fc_bf("p_xv"), fc_bf("g_xv")
        gt_xv, pt_xv, q_xv = fc_bf("gt_xv"), fc_bf("pt_xv"), fc_bf("q_xv")
        zt_xv, mm_xv = fc_bf("zt_xv"), fc_bf("mm_xv")
        sc1, sc2, sc3 = fc_bf("sc1"), fc_bf("sc2"), fc_bf("sc3")
        lbc, bvbc = fc_bf("lbc"), fc_bf("bvbc")
        araw = sl_tile("araw")
        alpha4, lfhx, sxx = sl_tile("alpha4"), sl_tile("lfhx"), sl_tile("sxx")
        c0s, viol, nviol, infs = (sl_tile("c0s"), sl_tile("viol"),
                                  sl_tile("nviol"), sl_tile("infs"))
        nums, dens, lams = sl_tile("nums"), sl_tile("dens"), sl_tile("lams")
        t1s, t2s = sl_tile("t1s"), sl_tile("t2s")

        # ---------------- Phase A ----------------
        # px/alpha matmul outs leave pad regions unwritten; zero once so the
        # full-tile evac copies and transposes never see uninitialized PSUM.
        LPx = psA.tile([128, 2, 128], f32, tag="LPx", name="LPx")
        alP = psA.tile([128, 2, 128], f32, tag="alP", name="alP")
        V.memset(LPx[:], 0.0)
        V.memset(alP[:], 0.0)
        for t in range(NT):
            cs = slice(128 * t, 128 * t + 128)
            ss = slice(16 * t, 16 * t + 16)
            nc.sync.dma_start(
                x_xv[:, cs].rearrange("p (b s j) -> p b s j", b=8, s=2, j=8),
                x_d[t * TILE:(t + 1) * TILE, :].rearrange(
                    "(b r s) j -> r b s j", b=8, r=128, s=2))
            TP = psA.tile([128, 3, 128], f32, tag="TP", name="TP")
            nc.tensor.transpose(TP[:, 0, :], x_xv[:, cs], C["ID128"][:])
            xsp2 = work.tile([128, 128], bf16, tag="xsp2", name="xsp2")
            V.tensor_copy(xsp2[:], TP[:, 0, :])

            h1P = psA.tile([128, 4, 128], f32, tag="h1P", name="h1P")
            x2P = psA.tile([128, 4, 128], f32, tag="x2P", name="x2P")
            h1 = work.tile([128, 8, 128], bf16, tag="h1", name="h1")
            x2 = work.tile([128, 8, 128], bf16, tag="x2", name="x2")

            for half in range(2):
                hs = slice(4 * half, 4 * half + 4)
                for bi in range(4):
                    b = 4 * half + bi
                    nc.tensor.matmul(h1P[:, bi, :], C[f"TL1E{b}"][:], xsp2[:])
                # relu + bias evac, one wide op per half (PSUM: DVE/ACT only)
                SC.activation(h1[:, hs, :], h1P[:], AF.Relu, bias=C["B1v"][:])
                for bi in range(4):
                    b = 4 * half + bi
                    nc.tensor.matmul(x2P[:, bi, :], C["TL2"][:], h1[:, b, :])
                SC.activation(x2[:, hs, :], x2P[:], AF.Relu, bias=C["B2v"][:])
                for bi in range(4):
                    b = 4 * half + bi
                    m4, k2 = b % 4, b // 4
                    nc.tensor.matmul(LPx[32 * m4:32 * m4 + 16, k2, :],
                                     C["TL3px"][:], x2[0:64, b, :],
                                     tile_position=(0, 32 * m4))
                    nc.tensor.matmul(alP[32 * m4:32 * m4 + 2, k2, :],
                                     C["TL3a"][64:128, :], x2[64:128, b, :],
                                     tile_position=(64, 32 * m4))

            # raw px / alpha evac (bias+sigmoid deferred to xview)
            pxe = work.tile([128, 2, 128], bf16, tag="pxe", name="pxe")
            asle = work.tile([128, 2, 128], bf16, tag="asle", name="asle")
            SC.activation(pxe[:], LPx[:], AF.Copy)
            SC.activation(asle[:], alP[:], AF.Copy)

            nc.tensor.matmul(TP[:, 1, :], C["TDA"][:], xsp2[:])
            nc.tensor.matmul(TP[:, 2, :], C["TDG"][:], xsp2[:])
            axs = work.tile([128, 128], bf16, tag="axs", name="axs")
            gsp2 = work.tile([128, 128], bf16, tag="gsp2", name="gsp2")
            V.tensor_copy(axs[:], TP[:, 1, :])
            V.tensor_copy(gsp2[:], TP[:, 2, :])

            # transposes back to xview
            trP = psB.tile([128, 2, 128], bf16, tag="trP", name="trP")
            pxtP = psB.tile([128, 2, 128], bf16, tag="pxtP", name="pxtP")
            altP = psB.tile([128, 2, 128], bf16, tag="altP", name="altP")
            nc.tensor.transpose(trP[:, 0, :], gsp2[:], C["ID128H"][:])
            nc.tensor.transpose(trP[:, 1, :], axs[:], C["ID128H"][:])
            V.tensor_copy(g_xv[:, cs], trP[:, 0, :])
            prodA = work.tile([128, 128], f32, tag="prodA", name="prodA")
            V.scalar_tensor_tensor(prodA[:], trP[:, 1, :], -2.0,
                                   x_xv[:, cs], AL.mult, AL.mult)
            V.tensor_reduce(lfhx[:, ss],
                            prodA[:].rearrange("p (c j) -> p c j", j=8),
                            XL, AL.add)
            sqx = work.tile([128, 128], f32, tag="sqx", name="sqx")
            GP.tensor_tensor(sqx[:], x_xv[:, cs], x_xv[:, cs], AL.mult)
            V.tensor_reduce(sxx[:, ss],
                            sqx[:].rearrange("p (c j) -> p c j", j=8),
                            XL, AL.add)

            for k in range(2):
                nc.tensor.transpose(pxtP[:, k, :], pxe[:, k, :], C["ID128H"][:])
                nc.tensor.transpose(altP[:, k, :], asle[:, k, :], C["ID128H"][:])
            # px (+ b31 bias) and alpha back to xview, one wide op each
            dstp = p_xv[:, cs].rearrange("p (k m sj) -> p k m sj",
                                         k=2, m=4, sj=16)
            srcp = pxtP.rearrange("p k (m g sj) -> p k m g sj",
                                     m=4, g=2, sj=16)[:, :, :, 0, :]
            V.tensor_tensor(
                dstp, srcp,
                C["B31J"][:].rearrange("p (k m sj) -> p k m sj",
                                       k=1, m=1, sj=16)
                .broadcast_to((128, 2, 4, 16)), AL.add)
            dsta = araw[:, ss].rearrange("p (k m s) -> p k m s", k=2, m=4, s=2)
            srca = altP.rearrange("p k (m g) -> p k m g",
                                     m=4, g=32)[:, :, :, 0:2]
            V.tensor_copy(dsta, srca)

        # ---------------- Phase B (per chunk) ----------------
        CF = FC // NCH          # 1024 fc cols per chunk
        CL = NSLOT // NCH       # 128 slot cols per chunk
        x3 = lambda ap: ap.rearrange("p (c j) -> p c j", j=8)

        for ch in range(NCH):
            fs = slice(CF * ch, CF * ch + CF)
            sl = slice(CL * ch, CL * ch + CL)
            pF, gF = p_xv[:, fs], g_xv[:, fs]
            gtF, ptF, qF = gt_xv[:, fs], pt_xv[:, fs], q_xv[:, fs]
            ztF, mmF = zt_xv[:, fs], mm_xv[:, fs]
            s1F, s2F, s3F = sc1[:, fs], sc2[:, fs], sc3[:, fs]
            lbcF, bvbcF = lbc[:, fs], bvbc[:, fs]
            u32F = u32[:, fs]
            c0L, viL, nviL, inL = c0s[:, sl], viol[:, sl], nviol[:, sl], infs[:, sl]
            nmL, dnL, lmL = nums[:, sl], dens[:, sl], lams[:, sl]
            t1L, t2L = t1s[:, sl], t2s[:, sl]
            arL, a4L = araw[:, sl], alpha4[:, sl]
            bcv = lambda apL: apL.broadcast_to((128, CL, 8))

            # alpha sigmoid (deferred from phase A; b31 bias folded into evac)
            SC.activation(a4L, arL, AF.Sigmoid, bias=C["B32s"][:])

            # c0 = Lfhx + 4*sigm*(16 - sxx);  (alpha4 holds the sigmoid)
            GP.tensor_scalar(t1L, sxx[:, sl], -1.0, 16.0, AL.mult, AL.add)
            GP.tensor_tensor(t2L, a4L, t1L, AL.mult)
            GP.tensor_scalar(t2L, t2L, 4.0, None, AL.mult)
            GP.tensor_tensor(c0L, t2L, lfhx[:, sl], AL.add)

            # transformed QP data
            SC.sign(s1F, gF)                                   # sigma
            V.tensor_tensor(ptF, s1F, pF, AL.mult)             # pt = sigma*p
            V.tensor_scalar(ztF, ptF, -1.0, None, AL.mult)     # zt0 = -pt
            SC.activation(gtF, gF, AF.Abs)
            SC.activation(qF, gF, AF.Square)
            GP.memset(mmF, 1.0)

            # c(0) and feasibility
            V.tensor_scalar(s2F, ztF, 1.0, -1.0, AL.min, AL.max)   # u0
            V.tensor_tensor(s1F, gtF, s2F, AL.mult)
            V.tensor_reduce(t1L, x3(s1F), XL, AL.add)
            GP.tensor_tensor(t1L, c0L, t1L, AL.add)
            GP.tensor_scalar(viL, t1L, 0.0, None, AL.is_lt)
            GP.tensor_scalar(nviL, viL, -1.0, None, AL.mult)
            V.tensor_reduce(t2L, x3(gtF), XL, AL.add)
            GP.tensor_tensor(t2L, c0L, t2L, AL.add)
            GP.tensor_scalar(inL, t2L, 0.0, None, AL.is_lt)
            GP.tensor_tensor(inL, inL, viL, AL.mult)

            # initial num/den (zt = -pt, mm = 1)
            V.tensor_tensor(s1F, gtF, ztF, AL.mult)
            V.tensor_reduce(nmL, x3(s1F), XL, AL.add)
            GP.tensor_tensor(nmL, c0L, nmL, AL.add)
            V.tensor_reduce(dnL, x3(qF), XL, AL.add)

            def calc_lam():
                GP.tensor_scalar(t1L, dnL, EPS, None, AL.add)
                V.reciprocal(t2L, t1L)
                GP.tensor_tensor(lmL, nmL, t2L, AL.mult)
                GP.tensor_tensor(lmL, lmL, nviL, AL.mult)      # lam = -num/den*viol

            calc_lam()
            for _ in range(T_KIWIEL):
                SC.activation(x3(lbcF), bcv(lmL), AF.Copy)         # lam bcast
                V.tensor_tensor(s2F, lbcF, gtF, AL.mult)
                V.tensor_tensor(s2F, s2F, ptF, AL.subtract)        # ur
                V.tensor_scalar(s2F, s2F, 1.0, -1.0, AL.min, AL.max)  # uhat
                V.tensor_tensor(s1F, gtF, s2F, AL.mult)
                V.tensor_reduce(t1L, x3(s1F), XL, AL.add)
                GP.tensor_tensor(t1L, c0L, t1L, AL.add)            # c
                GP.tensor_scalar(t2L, t1L, -1.0, None, AL.mult)    # -c
                SC.activation(x3(bvbcF), bcv(t2L), AF.Sign)        # bvs = sign(-c)
                V.tensor_tensor(s1F, bvbcF, s2F, AL.mult)
                V.tensor_scalar(s1F, s1F, 1.0, None, AL.is_ge)
                V.tensor_tensor(s1F, s1F, mmF, AL.mult)            # fix
                GP.tensor_tensor(s3F, bvbcF, ztF, AL.subtract)
                V.tensor_tensor(s3F, s1F, s3F, AL.mult)
                V.tensor_tensor(ztF, ztF, s3F, AL.add)
                GP.tensor_tensor(mmF, mmF, s1F, AL.subtract)
                V.tensor_tensor(s1F, gtF, ztF, AL.mult)
                V.tensor_reduce(nmL, x3(s1F), XL, AL.add)
                GP.tensor_tensor(nmL, c0L, nmL, AL.add)
                V.tensor_tensor(s1F, qF, mmF, AL.mult)
                V.tensor_reduce(dnL, x3(s1F), XL, AL.add)
                calc_lam()

            # infeasible rows -> lam = LAMCAP
            GP.tensor_scalar(t1L, lmL, -1.0, LAMCAP, AL.mult, AL.add)
            GP.tensor_tensor(t1L, t1L, inL, AL.mult)
            GP.tensor_tensor(lmL, lmL, t1L, AL.add)
            SC.activation(x3(lbcF), bcv(lmL), AF.Copy)
            V.tensor_tensor(s1F, lbcF, gF, AL.mult)
            V.tensor_tensor(s1F, s1F, pF, AL.subtract)
            V.tensor_scalar(u32F, s1F, 1.0, -1.0, AL.min, AL.max)
            for tt in range(TPC * ch, TPC * (ch + 1)):
                nc.sync.dma_start(
                    u_d[tt * TILE:(tt + 1) * TILE, :].rearrange(
                        "(b r s) j -> r b s j", b=8, r=128, s=2),
                    u32[:, 128 * tt:128 * tt + 128].rearrange(
                        "p (b s j) -> p b s j", b=8, s=2, j=8))


def _build():
    from concourse import bacc, mybir
    from concourse import tile as tile_mod
    from concourse._compat import axon_active
    f32 = mybir.dt.float32
    bf16 = mybir.dt.bfloat16
    nc = bacc.Bacc("TRN2", target_bir_lowering=False,
                   debug=not axon_active(), num_devices=NCORES)
    x_d = nc.dram_tensor("x", [S, N], f32, kind="ExternalInput").ap()
    u_d = nc.dram_tensor("u", [S, N], f32, kind="ExternalOutput").ap()
    cds = {}
    for k, v in _CSHAPES_BF.items():
        cds[k] = nc.dram_tensor(k, list(v), bf16, kind="ExternalInput").ap()
    for k, v in _CSHAPES_F32.items():
        cds[k] = nc.dram_tensor(k, list(v), f32, kind="ExternalInput").ap()
    with tile_mod.TileContext(nc) as tc:
        build_kernel(nc, tc, x_d, u_d, cds)
    nc.compile()
    return nc


def kernel(x, W1, b1, W21, b21, W22, b22, W31, b31, W32, b32, A, G, mean, std):
    from concourse.bass_utils import run_bass_kernel_spmd
    f32 = np.float32
    x = np.asarray(x, f32)
    x0 = (x * np.asarray(std, f32) + np.asarray(mean, f32)).astype(f32)

    consts = _consts(np.asarray(W1, f32), np.asarray(b1, f32), np.asarray(W21, f32),
                     np.asarray(b21, f32), np.asarray(W22, f32), np.asarray(b22, f32),
                     np.asarray(W31, f32), np.asarray(b31, f32), np.asarray(W32, f32),
                     np.asarray(b32, f32), np.asarray(A, f32), np.asarray(G, f32))
    if "nc" not in _CACHE:
        _CACHE["nc"] = _build()
    nc = _CACHE["nc"]

    in_maps = []
    for c in range(NCORES):
        m = {"x": np.ascontiguousarray(x0[c * S:(c + 1) * S])}
        m.update(consts)
        in_maps.append(m)
    res = run_bass_kernel_spmd(nc, in_maps, list(range(NCORES)))
    out = np.concatenate([np.asarray(res.results[c]["u"]) for c in range(NCORES)],
                         axis=0)
    return out.astype(f32)
